# revision 16
# baseline (speedup 1.0000x reference)
"""GCN encoder (3x gcn_conv) on 8 Trainium2 NeuronCores.

Graph-parallel by destination node; per-core edges grouped into 128-node
destination blocks. Key structure (v2):

- Self-loop terms are folded in as ordinary edges: the edge-attr matrix gets a
  9th row that selects a (bias+root) row of the augmented edge weights, and
  srw=dinv[v] makes the 1/deg self scaling come out after the per-block
  dinv[col] scale.
- Node tables are bf16 in DRAM and fetched with batched dma_gather
  (transpose=True), so each 128-edge chunk's table values arrive as a
  pre-transposed [feat, edge] tile. int16 gather indices limit a table to
  32768 rows, so tables are split low/high and each block's edges are
  reordered into a low run then a high run (each padded to 128).
- Per chunk: two accumulating bf16 matmuls build relu-input PSUM
  (gathered-table @ W + at9 @ weaug), ACT applies relu into a bf16 msg tile,
  DVE builds the one-hot scatter matrix S[e,dst] = (iota==colrel)*dinv_row,
  and a third matmul accumulates S^T @ msg into the block's PSUM agg.
- Block final: relu(dinv[col] * agg) in one ACT op -> h block; T2 = h @
  [Wmu|Wls] per own block; ONE bf16 AllGather of T2 between the passes.
  Layer-2 repeats the edge pass over the T2 table (identity rhs) and writes
  dinv[col] * agg2 as the output.
"""
import numpy as np

N_NODES = 50000
N_CORES = 8
SHARD = N_NODES // N_CORES          # 6250
P = 128
NBLK = (SHARD + P - 1) // P         # 49 destination blocks / core
IN_F = 128
HID = 128
OUT_F = 64
TSPLIT = 32768                      # int16 gather index limit
TBLK = (N_NODES + P - 1) // P       # 391 x-cast chunks


def _host_prep(x, edge_index, edge_attr,
               W1, b1, We1, be1, root1,
               Wmu, bmu, Wemu, bemu, rootmu,
               Wls, bls, Wels, bels, rootls):
    x = np.asarray(x, np.float32)
    row = np.asarray(edge_index[0], np.int64)
    col = np.asarray(edge_index[1], np.int64)
    ea = np.asarray(edge_attr, np.float32)
    E = row.shape[0]

    deg = (np.bincount(row, minlength=N_NODES) + 1.0).astype(np.float32)
    dinv = deg ** -0.5

    # append self-edges (v -> v); at row 8 marks them
    selfv = np.arange(N_NODES, dtype=np.int64)
    row_a = np.concatenate([row, selfv])
    col_a = np.concatenate([col, selfv])
    is_self = np.concatenate([np.zeros(E, bool), np.ones(N_NODES, bool)])
    EA = row_a.shape[0]

    core_of = col_a // SHARD
    blk_of = (col_a - core_of * SHARD) // P
    cls_of = (row_a >= TSPLIT).astype(np.int64)   # 0 = low table, 1 = high

    # uniform chunks per (block, class) across cores
    counts = np.zeros((N_CORES, NBLK, 2), np.int64)
    for c in range(N_CORES):
        m = core_of == c
        np.add.at(counts[c], (blk_of[m], cls_of[m]), 1)
    ncls = (counts.max(axis=0) + P - 1) // P      # [NBLK, 2]
    NCH = int(ncls.sum())

    # chunk-column base for each (block, class) run; runs laid out
    # block-major, low run then high run
    run_base = np.zeros((NBLK, 2), np.int64)
    acc = 0
    runs = []          # (b, cls, nch, chunk_base)
    for b in range(NBLK):
        for cl in range(2):
            run_base[b, cl] = acc
            if ncls[b, cl] > 0:
                runs.append((b, cl, int(ncls[b, cl]), acc))
            acc += int(ncls[b, cl])

    colrel = np.full((N_CORES, P, NCH), -1.0, np.float32)
    srw = np.zeros((N_CORES, P, NCH), np.float32)
    at = np.zeros((N_CORES, 9, NCH * P), np.float32)
    gidx = np.zeros((N_CORES, P, NCH * 8), np.int16)

    order = np.lexsort((cls_of, blk_of, core_of))
    row_s, col_s = row_a[order], col_a[order]
    core_s, blk_s, cls_s = core_of[order], blk_of[order], cls_of[order]
    self_s = is_self[order]
    ea_idx = order  # indices into the augmented edge list; < E means real

    seg_cnt = np.zeros(N_CORES * NBLK * 2 + 1, np.int64)
    seg_key = (core_s * NBLK + blk_s) * 2 + cls_s
    np.add.at(seg_cnt, seg_key + 1, 1)
    seg_start = np.cumsum(seg_cnt)
    pos_in_seg = np.arange(EA) - seg_start[seg_key]

    chunk_idx = run_base[blk_s, cls_s] + pos_in_seg // P
    part_idx = pos_in_seg % P

    colrel[core_s, part_idx, chunk_idx] = (col_s - core_s * SHARD - blk_s * P).astype(np.float32)
    srw[core_s, part_idx, chunk_idx] = dinv[row_s]
    flat = chunk_idx * P + part_idx
    real = ~self_s
    for j in range(7):
        at[core_s[real], j, flat[real]] = ea[ea_idx[real], j]
    at[core_s[real], 7, flat[real]] = 1.0
    at[core_s[self_s], 8, flat[self_s]] = 1.0

    # gather indices, rebased per table half, wrapped-16 layout replicated x8:
    # index position i within a run -> [16g + i%16, run_chunk_base*8 + i//16]
    rel_row = (row_s - cls_s * TSPLIT).astype(np.int16)
    gcol = run_base[blk_s, cls_s] * 8 + pos_in_seg // 16
    gpart = pos_in_seg % 16
    for g in range(8):
        gidx[core_s, 16 * g + gpart, gcol] = rel_row

    dinvcol = np.zeros((N_CORES, P, NBLK), np.float32)
    for c in range(N_CORES):
        ids = c * SHARD + np.arange(SHARD)
        b = np.arange(SHARD) // P
        p = np.arange(SHARD) % P
        dinvcol[c, p, b] = dinv[ids]

    def bf(a):
        import ml_dtypes
        return np.asarray(a, np.float32).astype(ml_dtypes.bfloat16)

    weaug1 = np.concatenate([np.asarray(We1, np.float32),
                             (np.asarray(be1) + np.asarray(b1))[None, :],
                             (np.asarray(b1) + np.asarray(root1))[None, :]], 0)
    weaug2 = np.concatenate([
        np.concatenate([np.asarray(Wemu), np.asarray(Wels)], 1),
        np.concatenate([np.asarray(bemu) + np.asarray(bmu),
                        np.asarray(bels) + np.asarray(bls)])[None, :],
        np.concatenate([np.asarray(bmu) + np.asarray(rootmu),
                        np.asarray(bls) + np.asarray(rootls)])[None, :]], 0)
    wcat = np.concatenate([np.asarray(Wmu), np.asarray(Wls)], 1)
    iota = np.tile(np.arange(P, dtype=np.float32)[None, :], (P, 1))
    ident = np.eye(P, dtype=np.float32)

    shared = dict(xrows=np.ascontiguousarray(x), W1b=bf(W1), wcatb=bf(wcat),
                  we1b=bf(weaug1), we2b=bf(weaug2), iotab=bf(iota),
                  identb=bf(ident))
    per_core = []
    for c in range(N_CORES):
        d = dict(colrel=colrel[c], srw=srw[c], at=bf(at[c]), gidx=gidx[c],
                 dinvcol=dinvcol[c])
        d.update(shared)
        per_core.append(d)
    return per_core, runs, NCH


def _build_nc(runs, NCH):
    from concourse import bass, bacc, mybir
    import concourse.tile as tile

    f32 = mybir.dt.float32
    bf16 = mybir.dt.bfloat16
    i16 = mybir.dt.int16
    Relu = mybir.ActivationFunctionType.Relu
    Copy = mybir.ActivationFunctionType.Copy
    Alu = mybir.AluOpType
    nc = bacc.Bacc(None, num_devices=N_CORES)

    xrows_d = nc.declare_dram_parameter("xrows", [N_NODES, IN_F], f32, isOutput=False)
    W1b_d = nc.declare_dram_parameter("W1b", [IN_F, HID], bf16, isOutput=False)
    wcatb_d = nc.declare_dram_parameter("wcatb", [HID, P], bf16, isOutput=False)
    we1b_d = nc.declare_dram_parameter("we1b", [9, HID], bf16, isOutput=False)
    we2b_d = nc.declare_dram_parameter("we2b", [9, P], bf16, isOutput=False)
    iotab_d = nc.declare_dram_parameter("iotab", [P, P], bf16, isOutput=False)
    identb_d = nc.declare_dram_parameter("identb", [P, P], bf16, isOutput=False)
    colrel_d = nc.declare_dram_parameter("colrel", [P, NCH], f32, isOutput=False)
    srw_d = nc.declare_dram_parameter("srw", [P, NCH], f32, isOutput=False)
    at_d = nc.declare_dram_parameter("at", [9, NCH * P], bf16, isOutput=False)
    gidx_d = nc.declare_dram_parameter("gidx", [P, NCH * 8], i16, isOutput=False)
    dinvcol_d = nc.declare_dram_parameter("dinvcol", [P, NBLK], f32, isOutput=False)
    out_d = nc.declare_dram_parameter("out", [SHARD, P], f32, isOutput=True)

    xb_dram = nc.dram_tensor("xb", [N_NODES, IN_F], bf16)
    t2shard = nc.dram_tensor("t2shard", [SHARD, P], bf16)
    t2full = nc.dram_tensor("t2full", [N_NODES, P], bf16, addr_space="Shared")

    SUP = 4      # chunks per relu batch
    ATSUP = 64   # chunks per at-stream tile
    MAXRUN = max(n for (_b, _cl, n, _cb) in runs)

    # per-chunk schedule: (b, cls, k_in_run, run_chunk_base, first, last)
    sched = []
    for b in range(NBLK):
        blk_runs = [r for r in runs if r[0] == b]
        tot = sum(r[2] for r in blk_runs)
        ki = 0
        for (_b, cl, n, cb) in blk_runs:
            for k in range(n):
                sched.append((b, cl, k, cb, ki == 0, ki == tot - 1))
                ki += 1

    with tile.TileContext(nc) as tc:
        with (
            tc.tile_pool(name="const", bufs=1) as cpool,
            tc.tile_pool(name="stream", bufs=2) as stpool,
            tc.tile_pool(name="gat", bufs=3) as gpool,
            tc.tile_pool(name="work", bufs=3) as wpool,
            tc.tile_pool(name="node", bufs=3) as npool,
            tc.tile_pool(name="cast", bufs=4) as xpool,
            tc.tile_pool(name="pse", bufs=2, space="PSUM") as pse,
            tc.tile_pool(name="psagg", bufs=2, space="PSUM") as psagg,
            tc.tile_pool(name="psnode", bufs=2, space="PSUM") as psnode,
        ):
            W1b_t = cpool.tile([IN_F, HID], bf16)
            wcatb_t = cpool.tile([HID, P], bf16)
            we1b_t = cpool.tile([9, HID], bf16)
            we2b_t = cpool.tile([9, P], bf16)
            iotab_t = cpool.tile([P, P], bf16)
            identb_t = cpool.tile([P, P], bf16)
            colrel_t = cpool.tile([P, NCH], f32)
            srw_t = cpool.tile([P, NCH], f32)
            gidx_t = cpool.tile([P, NCH * 8], i16)
            dinvcol_t = cpool.tile([P, NBLK], f32)
            for t, d in ((W1b_t, W1b_d), (wcatb_t, wcatb_d), (we1b_t, we1b_d),
                         (we2b_t, we2b_d), (iotab_t, iotab_d),
                         (identb_t, identb_d), (colrel_t, colrel_d),
                         (srw_t, srw_d), (gidx_t, gidx_d),
                         (dinvcol_t, dinvcol_d)):
                nc.sync.dma_start(out=t[:], in_=d[:])

            # ---- phase 1: cast x -> bf16 table in DRAM (8 chunks per DMA) ----
            CB = 8
            for tb in range(0, TBLK, CB):
                nchk = min(CB, TBLK - tb)
                lo = tb * P
                n = min(nchk * P, N_NODES - lo)
                xt = xpool.tile([P, CB, IN_F], f32, tag="xt")
                xbt = xpool.tile([P, CB, IN_F], bf16, tag="xbt")
                full = n // P          # whole chunks in this batch
                if full:
                    xv = xrows_d[lo:lo + full * P, :].rearrange("(c p) f -> p c f", p=P)
                    nc.sync.dma_start(out=xt[:, :full, :], in_=xv)
                    nc.vector.tensor_copy(out=xbt[:, :full, :], in_=xt[:, :full, :])
                    nc.sync.dma_start(
                        out=xb_dram[lo:lo + full * P, :].rearrange("(c p) f -> p c f", p=P),
                        in_=xbt[:, :full, :])
                rem = n - full * P     # trailing partial chunk (last batch)
                if rem:
                    xr = xpool.tile([P, IN_F], f32, tag="xtr")
                    nc.sync.dma_start(out=xr[:rem, :], in_=xrows_d[lo + full * P:lo + n, :])
                    xbr = xpool.tile([P, IN_F], bf16, tag="xbr")
                    nc.vector.tensor_copy(out=xbr[:rem, :], in_=xr[:rem, :])
                    nc.sync.dma_start(out=xb_dram[lo + full * P:lo + n, :], in_=xbr[:rem, :])

            # ---- edge pass ----
            # inject="mm": transposed gather + per-chunk table matmul (l1)
            # inject="add": plain gather + one wide DVE add per super (l2,
            #   whose table T2 is premultiplied so only the at-term needs PE)
            def edge_pass(layer, tab_lo, tab_hi, wtab_t, weaug_t, block_done,
                          inject):
                at_tile = [None]
                cur_at = [-1]
                sup = {}
                pend = []
                gt = {}

                def flush(nq):
                    if inject == "add":
                        pre = wpool.tile([P, SUP, P], f32, name="pre", tag="pre")
                        k0 = sup["k0"]
                        nc.vector.tensor_tensor(
                            out=pre[:, :nq, :],
                            in0=gt[sup["cl"]][:, k0:k0 + nq, :],
                            in1=sup["eps"][:, :nq, :], op=Alu.add)
                        nc.scalar.activation(sup["msg"][:, :nq, :],
                                             pre[:, :nq, :], Relu)
                    else:
                        nc.scalar.activation(sup["msg"][:, :nq, :],
                                             sup["eps"][:, :nq, :], Relu)
                    for (qq, bb, first, last, agg) in pend:
                        nc.tensor.matmul(
                            out=agg[:], lhsT=sup["S"][:, qq, :],
                            rhs=sup["msg"][:, qq, :],
                            start=first, stop=last)
                        if last:
                            block_done(bb, agg)
                    pend.clear()

                agg = None
                q = 0
                for cidx, (b, cl, k, cb, first, last) in enumerate(sched):
                    if q == SUP or (inject == "add" and k == 0 and q > 0):
                        flush(q)
                        q = 0
                    if k == 0:
                        # new run: batched gathers (the SWDGE ring wedges
                        # beyond 896 indices per instruction)
                        GMAX = 7
                        n = next(r[2] for r in runs if r[0] == b and r[1] == cl)
                        if inject == "mm":
                            gt[cl] = gpool.tile([P, 1, MAXRUN * P], bf16,
                                                name=f"g{layer}c{cl}", tag=f"g{cl}")
                        else:
                            gt[cl] = gpool.tile([P, MAXRUN, P], bf16,
                                                name=f"g{layer}c{cl}", tag=f"g{cl}")
                        for s0 in range(0, n, GMAX):
                            sn = min(GMAX, n - s0)
                            ni = sn * P
                            out_ap = (gt[cl][:, :, s0 * P:s0 * P + ni]
                                      if inject == "mm"
                                      else gt[cl][:, s0:s0 + sn, :])
                            nc.gpsimd.dma_gather(
                                out_ap=out_ap,
                                in_ap=tab_lo[:] if cl == 0 else tab_hi[:],
                                idxs_ap=gidx_t[:, (cb + s0) * 8:(cb + s0) * 8 + ni // 16],
                                num_idxs=ni, num_idxs_reg=ni,
                                elem_size=P, transpose=(inject == "mm"))
                    if q == 0:
                        sup["eps"] = pse.tile([P, SUP, P], f32, name="eps", tag="eps")
                        sup["S"] = wpool.tile([P, SUP, P], bf16, name=f"S{layer}", tag="S")
                        sup["msg"] = wpool.tile([P, SUP, P], bf16, name=f"msg{layer}", tag="msg")
                        sup["k0"] = k
                        sup["cl"] = cl
                    if cidx // ATSUP != cur_at[0]:
                        cur_at[0] = cidx // ATSUP
                        lo = cur_at[0] * ATSUP * P
                        n2 = min(ATSUP * P, NCH * P - lo)
                        at_tile[0] = stpool.tile([9, ATSUP * P], bf16, name="at", tag="at")
                        nc.sync.dma_start(out=at_tile[0][:, :n2], in_=at_d[:, lo:lo + n2])
                    if first:
                        agg = psagg.tile([P, P], f32, tag="agg")
                    a0 = (cidx - cur_at[0] * ATSUP) * P
                    if inject == "mm":
                        nc.tensor.matmul(out=sup["eps"][:, q, :],
                                         lhsT=gt[cl][:, 0, k * P:(k + 1) * P],
                                         rhs=wtab_t[:], start=True, stop=False)
                        nc.tensor.matmul(out=sup["eps"][:, q, :],
                                         lhsT=at_tile[0][:, a0:a0 + P],
                                         rhs=weaug_t[:], start=False, stop=True)
                    else:
                        nc.tensor.matmul(out=sup["eps"][:, q, :],
                                         lhsT=at_tile[0][:, a0:a0 + P],
                                         rhs=weaug_t[:], start=True, stop=True)
                    nc.vector.tensor_scalar(
                        out=sup["S"][:, q, :], in0=iotab_t[:],
                        scalar1=colrel_t[:, cidx:cidx + 1],
                        scalar2=srw_t[:, cidx:cidx + 1],
                        op0=Alu.is_equal, op1=Alu.mult)
                    pend.append((q, b, first, last, agg))
                    q += 1
                    if cidx == len(sched) - 1:
                        flush(q)

            # ---- phase 2: layer-1 pass; block finals build T2 shard ----
            def l1_block_done(b, agg):
                hb = npool.tile([P, HID], bf16, tag="hb")
                nc.scalar.activation(hb[:], agg[:], Relu,
                                     scale=dinvcol_t[:, b:b + 1])
                pst = psnode.tile([P, P], bf16, tag="pnT")
                nc.tensor.transpose(out=pst[:], in_=hb[:], identity=identb_t[:])
                hbT = npool.tile([P, P], bf16, tag="hbT")
                nc.scalar.activation(hbT[:], pst[:], Copy)
                ps2 = psnode.tile([P, P], f32, tag="pn")
                nc.tensor.matmul(out=ps2[:], lhsT=hbT[:], rhs=wcatb_t[:],
                                 start=True, stop=True)
                t2b = npool.tile([P, P], bf16, tag="t2b")
                nc.scalar.activation(t2b[:], ps2[:], Copy)
                lo = b * P
                n = min(P, SHARD - lo)
                nc.sync.dma_start(out=t2shard[lo:lo + n, :], in_=t2b[:n, :])

            edge_pass(1, xb_dram[0:TSPLIT, :], xb_dram[TSPLIT:, :],
                      W1b_t, we1b_t, l1_block_done, inject="mm")

            # ---- phase 3: one AllGather of the T2 table ----
            nc.gpsimd.collective_compute(
                "AllGather", mybir.AluOpType.bypass,
                replica_groups=[list(range(N_CORES))],
                ins=[t2shard[:]], outs=[t2full[:]])

            # ---- phase 4: layer-2/3 pass ----
            def l2_block_done(b, agg):
                w = npool.tile([P, P], f32, tag="w2")
                nc.scalar.activation(w[:], agg[:], Copy,
                                     scale=dinvcol_t[:, b:b + 1])
                lo = b * P
                n = min(P, SHARD - lo)
                nc.sync.dma_start(out=out_d[lo:lo + n, :], in_=w[:n, :])

            edge_pass(2, t2full[0:TSPLIT, :], t2full[TSPLIT:, :],
                      None, we2b_t, l2_block_done, inject="add")

    nc.finalize()
    return nc


_CACHE = {}


def kernel(**inputs):
    from concourse.bass_utils import run_bass_kernel_spmd

    per_core, runs, NCH = _host_prep(**inputs)
    key = (tuple(map(tuple, runs)), NCH)
    if key not in _CACHE:
        _CACHE[key] = _build_nc(runs, NCH)
    nc = _CACHE[key]
    r = None
    for attempt in range(3):
        try:
            r = run_bass_kernel_spmd(nc, per_core, list(range(N_CORES)))
            break
        except Exception:
            if attempt == 2:
                raise
            import time as _time
            _time.sleep(5.0)
    outs = [r.results[c]["out"] for c in range(N_CORES)]
    full = np.concatenate(outs, axis=0)
    mu = np.ascontiguousarray(full[:, :OUT_F])
    logstd = np.ascontiguousarray(full[:, OUT_F:])
    return (mu, logstd)


# revision 17
# speedup vs baseline: 1.0040x; 1.0040x over previous
"""GCN encoder (3x gcn_conv) on 8 Trainium2 NeuronCores.

Graph-parallel by destination node; per-core edges grouped into 128-node
destination blocks. Key structure (v2):

- Self-loop terms are folded in as ordinary edges: the edge-attr matrix gets a
  9th row that selects a (bias+root) row of the augmented edge weights, and
  srw=dinv[v] makes the 1/deg self scaling come out after the per-block
  dinv[col] scale.
- Node tables are bf16 in DRAM and fetched with batched dma_gather
  (transpose=True), so each 128-edge chunk's table values arrive as a
  pre-transposed [feat, edge] tile. int16 gather indices limit a table to
  32768 rows, so tables are split low/high and each block's edges are
  reordered into a low run then a high run (each padded to 128).
- Per chunk: two accumulating bf16 matmuls build relu-input PSUM
  (gathered-table @ W + at9 @ weaug), ACT applies relu into a bf16 msg tile,
  DVE builds the one-hot scatter matrix S[e,dst] = (iota==colrel)*dinv_row,
  and a third matmul accumulates S^T @ msg into the block's PSUM agg.
- Block final: relu(dinv[col] * agg) in one ACT op -> h block; T2 = h @
  [Wmu|Wls] per own block; ONE bf16 AllGather of T2 between the passes.
  Layer-2 repeats the edge pass over the T2 table (identity rhs) and writes
  dinv[col] * agg2 as the output.
"""
import numpy as np

N_NODES = 50000
N_CORES = 8
SHARD = N_NODES // N_CORES          # 6250
P = 128
NBLK = (SHARD + P - 1) // P         # 49 destination blocks / core
IN_F = 128
HID = 128
OUT_F = 64
TSPLIT = 32768                      # int16 gather index limit
TBLK = (N_NODES + P - 1) // P       # 391 x-cast chunks


def _host_prep(x, edge_index, edge_attr,
               W1, b1, We1, be1, root1,
               Wmu, bmu, Wemu, bemu, rootmu,
               Wls, bls, Wels, bels, rootls):
    x = np.asarray(x, np.float32)
    row = np.asarray(edge_index[0], np.int64)
    col = np.asarray(edge_index[1], np.int64)
    ea = np.asarray(edge_attr, np.float32)
    E = row.shape[0]

    deg = (np.bincount(row, minlength=N_NODES) + 1.0).astype(np.float32)
    dinv = deg ** -0.5

    # append self-edges (v -> v); at row 8 marks them
    selfv = np.arange(N_NODES, dtype=np.int64)
    row_a = np.concatenate([row, selfv])
    col_a = np.concatenate([col, selfv])
    is_self = np.concatenate([np.zeros(E, bool), np.ones(N_NODES, bool)])
    EA = row_a.shape[0]

    core_of = col_a // SHARD
    blk_of = (col_a - core_of * SHARD) // P
    cls_of = (row_a >= TSPLIT).astype(np.int64)   # 0 = low table, 1 = high

    # uniform chunks per (block, class) across cores
    counts = np.zeros((N_CORES, NBLK, 2), np.int64)
    for c in range(N_CORES):
        m = core_of == c
        np.add.at(counts[c], (blk_of[m], cls_of[m]), 1)
    ncls = (counts.max(axis=0) + P - 1) // P      # [NBLK, 2]
    NCH = int(ncls.sum())

    # chunk-column base for each (block, class) run; runs laid out
    # block-major, low run then high run
    run_base = np.zeros((NBLK, 2), np.int64)
    acc = 0
    runs = []          # (b, cls, nch, chunk_base)
    for b in range(NBLK):
        for cl in range(2):
            run_base[b, cl] = acc
            if ncls[b, cl] > 0:
                runs.append((b, cl, int(ncls[b, cl]), acc))
            acc += int(ncls[b, cl])

    colrel = np.full((N_CORES, P, NCH), -1.0, np.float32)
    srw = np.zeros((N_CORES, P, NCH), np.float32)
    at = np.zeros((N_CORES, 9, NCH * P), np.float32)
    gidx = np.zeros((N_CORES, P, NCH * 8), np.int16)

    order = np.lexsort((cls_of, blk_of, core_of))
    row_s, col_s = row_a[order], col_a[order]
    core_s, blk_s, cls_s = core_of[order], blk_of[order], cls_of[order]
    self_s = is_self[order]
    ea_idx = order  # indices into the augmented edge list; < E means real

    seg_cnt = np.zeros(N_CORES * NBLK * 2 + 1, np.int64)
    seg_key = (core_s * NBLK + blk_s) * 2 + cls_s
    np.add.at(seg_cnt, seg_key + 1, 1)
    seg_start = np.cumsum(seg_cnt)
    pos_in_seg = np.arange(EA) - seg_start[seg_key]

    chunk_idx = run_base[blk_s, cls_s] + pos_in_seg // P
    part_idx = pos_in_seg % P

    colrel[core_s, part_idx, chunk_idx] = (col_s - core_s * SHARD - blk_s * P).astype(np.float32)
    srw[core_s, part_idx, chunk_idx] = dinv[row_s]
    flat = chunk_idx * P + part_idx
    real = ~self_s
    for j in range(7):
        at[core_s[real], j, flat[real]] = ea[ea_idx[real], j]
    at[core_s[real], 7, flat[real]] = 1.0
    at[core_s[self_s], 8, flat[self_s]] = 1.0

    # gather indices, rebased per table half, wrapped-16 layout replicated x8:
    # index position i within a run -> [16g + i%16, run_chunk_base*8 + i//16]
    rel_row = (row_s - cls_s * TSPLIT).astype(np.int16)
    gcol = run_base[blk_s, cls_s] * 8 + pos_in_seg // 16
    gpart = pos_in_seg % 16
    for g in range(8):
        gidx[core_s, 16 * g + gpart, gcol] = rel_row

    dinvcol = np.zeros((N_CORES, P, NBLK), np.float32)
    for c in range(N_CORES):
        ids = c * SHARD + np.arange(SHARD)
        b = np.arange(SHARD) // P
        p = np.arange(SHARD) % P
        dinvcol[c, p, b] = dinv[ids]

    def bf(a):
        import ml_dtypes
        return np.asarray(a, np.float32).astype(ml_dtypes.bfloat16)

    weaug1 = np.concatenate([np.asarray(We1, np.float32),
                             (np.asarray(be1) + np.asarray(b1))[None, :],
                             (np.asarray(b1) + np.asarray(root1))[None, :]], 0)
    weaug2 = np.concatenate([
        np.concatenate([np.asarray(Wemu), np.asarray(Wels)], 1),
        np.concatenate([np.asarray(bemu) + np.asarray(bmu),
                        np.asarray(bels) + np.asarray(bls)])[None, :],
        np.concatenate([np.asarray(bmu) + np.asarray(rootmu),
                        np.asarray(bls) + np.asarray(rootls)])[None, :]], 0)
    wcat = np.concatenate([np.asarray(Wmu), np.asarray(Wls)], 1)
    iota = np.tile(np.arange(P, dtype=np.float32)[None, :], (P, 1))
    ident = np.eye(P, dtype=np.float32)

    shared = dict(xrows=np.ascontiguousarray(x), W1b=bf(W1), wcatb=bf(wcat),
                  we1b=bf(weaug1), we2b=bf(weaug2), iotab=bf(iota),
                  identb=bf(ident))
    per_core = []
    for c in range(N_CORES):
        d = dict(colrel=colrel[c], srw=srw[c], at=bf(at[c]), gidx=gidx[c],
                 dinvcol=dinvcol[c])
        d.update(shared)
        per_core.append(d)
    return per_core, runs, NCH


def _build_nc(runs, NCH):
    from concourse import bass, bacc, mybir
    import concourse.tile as tile

    f32 = mybir.dt.float32
    bf16 = mybir.dt.bfloat16
    i16 = mybir.dt.int16
    Relu = mybir.ActivationFunctionType.Relu
    Copy = mybir.ActivationFunctionType.Copy
    Alu = mybir.AluOpType
    nc = bacc.Bacc(None, num_devices=N_CORES)

    xrows_d = nc.declare_dram_parameter("xrows", [N_NODES, IN_F], f32, isOutput=False)
    W1b_d = nc.declare_dram_parameter("W1b", [IN_F, HID], bf16, isOutput=False)
    wcatb_d = nc.declare_dram_parameter("wcatb", [HID, P], bf16, isOutput=False)
    we1b_d = nc.declare_dram_parameter("we1b", [9, HID], bf16, isOutput=False)
    we2b_d = nc.declare_dram_parameter("we2b", [9, P], bf16, isOutput=False)
    iotab_d = nc.declare_dram_parameter("iotab", [P, P], bf16, isOutput=False)
    identb_d = nc.declare_dram_parameter("identb", [P, P], bf16, isOutput=False)
    colrel_d = nc.declare_dram_parameter("colrel", [P, NCH], f32, isOutput=False)
    srw_d = nc.declare_dram_parameter("srw", [P, NCH], f32, isOutput=False)
    at_d = nc.declare_dram_parameter("at", [9, NCH * P], bf16, isOutput=False)
    gidx_d = nc.declare_dram_parameter("gidx", [P, NCH * 8], i16, isOutput=False)
    dinvcol_d = nc.declare_dram_parameter("dinvcol", [P, NBLK], f32, isOutput=False)
    out_d = nc.declare_dram_parameter("out", [SHARD, P], f32, isOutput=True)

    xb_dram = nc.dram_tensor("xb", [N_NODES, IN_F], bf16)
    t2shard = nc.dram_tensor("t2shard", [SHARD, P], bf16)
    t2full = nc.dram_tensor("t2full", [N_NODES, P], bf16, addr_space="Shared")

    SUP = 4      # chunks per relu batch
    ATSUP = 64   # chunks per at-stream tile
    MAXRUN = max(n for (_b, _cl, n, _cb) in runs)

    # per-chunk schedule: (b, cls, k_in_run, run_chunk_base, first, last)
    sched = []
    for b in range(NBLK):
        blk_runs = [r for r in runs if r[0] == b]
        tot = sum(r[2] for r in blk_runs)
        ki = 0
        for (_b, cl, n, cb) in blk_runs:
            for k in range(n):
                sched.append((b, cl, k, cb, ki == 0, ki == tot - 1))
                ki += 1

    with tile.TileContext(nc) as tc:
        with (
            tc.tile_pool(name="const", bufs=1) as cpool,
            tc.tile_pool(name="stream", bufs=2) as stpool,
            tc.tile_pool(name="gat", bufs=2) as gpool,
            tc.tile_pool(name="work", bufs=3) as wpool,
            tc.tile_pool(name="node", bufs=3) as npool,
            tc.tile_pool(name="cast", bufs=4) as xpool,
            tc.tile_pool(name="pse", bufs=2, space="PSUM") as pse,
            tc.tile_pool(name="psagg", bufs=2, space="PSUM") as psagg,
            tc.tile_pool(name="psnode", bufs=2, space="PSUM") as psnode,
        ):
            W1b_t = cpool.tile([IN_F, HID], bf16)
            wcatb_t = cpool.tile([HID, P], bf16)
            we1b_t = cpool.tile([9, HID], bf16)
            we2b_t = cpool.tile([9, P], bf16)
            iotab_t = cpool.tile([P, P], bf16)
            identb_t = cpool.tile([P, P], bf16)
            colrel_t = cpool.tile([P, NCH], f32)
            srw_t = cpool.tile([P, NCH], f32)
            gidx_t = cpool.tile([P, NCH * 8], i16)
            dinvcol_t = cpool.tile([P, NBLK], f32)
            for t, d in ((W1b_t, W1b_d), (wcatb_t, wcatb_d), (we1b_t, we1b_d),
                         (we2b_t, we2b_d), (iotab_t, iotab_d),
                         (identb_t, identb_d), (colrel_t, colrel_d),
                         (srw_t, srw_d), (gidx_t, gidx_d),
                         (dinvcol_t, dinvcol_d)):
                nc.sync.dma_start(out=t[:], in_=d[:])

            # ---- phase 1: cast x -> bf16 table in DRAM (8 chunks per DMA) ----
            CB = 8
            for tb in range(0, TBLK, CB):
                nchk = min(CB, TBLK - tb)
                lo = tb * P
                n = min(nchk * P, N_NODES - lo)
                xt = xpool.tile([P, CB, IN_F], f32, tag="xt")
                xbt = xpool.tile([P, CB, IN_F], bf16, tag="xbt")
                full = n // P          # whole chunks in this batch
                if full:
                    xv = xrows_d[lo:lo + full * P, :].rearrange("(c p) f -> p c f", p=P)
                    nc.sync.dma_start(out=xt[:, :full, :], in_=xv)
                    nc.vector.tensor_copy(out=xbt[:, :full, :], in_=xt[:, :full, :])
                    nc.sync.dma_start(
                        out=xb_dram[lo:lo + full * P, :].rearrange("(c p) f -> p c f", p=P),
                        in_=xbt[:, :full, :])
                rem = n - full * P     # trailing partial chunk (last batch)
                if rem:
                    xr = xpool.tile([P, IN_F], f32, tag="xtr")
                    nc.sync.dma_start(out=xr[:rem, :], in_=xrows_d[lo + full * P:lo + n, :])
                    xbr = xpool.tile([P, IN_F], bf16, tag="xbr")
                    nc.vector.tensor_copy(out=xbr[:rem, :], in_=xr[:rem, :])
                    nc.sync.dma_start(out=xb_dram[lo + full * P:lo + n, :], in_=xbr[:rem, :])

            # ---- edge pass ----
            # inject="mm": transposed gather + per-chunk table matmul (l1)
            # inject="add": plain gather + one wide DVE add per super (l2,
            #   whose table T2 is premultiplied so only the at-term needs PE)
            def edge_pass(layer, tab_lo, tab_hi, wtab_t, weaug_t, block_done,
                          inject):
                at_tile = [None]
                cur_at = [-1]
                sup = {}
                pend = []
                gt = {}

                def flush(nq):
                    if inject == "add":
                        pre = wpool.tile([P, SUP, P], f32, name="pre", tag="pre")
                        k0 = sup["k0"]
                        nc.vector.tensor_tensor(
                            out=pre[:, :nq, :],
                            in0=gt[sup["cl"]][:, k0:k0 + nq, :],
                            in1=sup["eps"][:, :nq, :], op=Alu.add)
                        nc.scalar.activation(sup["msg"][:, :nq, :],
                                             pre[:, :nq, :], Relu)
                    else:
                        nc.scalar.activation(sup["msg"][:, :nq, :],
                                             sup["eps"][:, :nq, :], Relu)
                    for (qq, bb, first, last, agg) in pend:
                        nc.tensor.matmul(
                            out=agg[:], lhsT=sup["S"][:, qq, :],
                            rhs=sup["msg"][:, qq, :],
                            start=first, stop=last)
                        if last:
                            block_done(bb, agg)
                    pend.clear()

                agg = None
                q = 0
                for cidx, (b, cl, k, cb, first, last) in enumerate(sched):
                    if q == SUP or (inject == "add" and k == 0 and q > 0):
                        flush(q)
                        q = 0
                    if k == 0:
                        # new run: batched gathers (the SWDGE ring wedges
                        # beyond 896 indices per instruction)
                        GMAX = 7
                        n = next(r[2] for r in runs if r[0] == b and r[1] == cl)
                        if inject == "mm":
                            gt[cl] = gpool.tile([P, 1, MAXRUN * P], bf16,
                                                name=f"g{layer}c{cl}", tag=f"g{cl}")
                        else:
                            gt[cl] = gpool.tile([P, MAXRUN, P], bf16,
                                                name=f"g{layer}c{cl}", tag=f"g{cl}")
                        for s0 in range(0, n, GMAX):
                            sn = min(GMAX, n - s0)
                            ni = sn * P
                            out_ap = (gt[cl][:, :, s0 * P:s0 * P + ni]
                                      if inject == "mm"
                                      else gt[cl][:, s0:s0 + sn, :])
                            nc.gpsimd.dma_gather(
                                out_ap=out_ap,
                                in_ap=tab_lo[:] if cl == 0 else tab_hi[:],
                                idxs_ap=gidx_t[:, (cb + s0) * 8:(cb + s0) * 8 + ni // 16],
                                num_idxs=ni, num_idxs_reg=ni,
                                elem_size=P, transpose=(inject == "mm"))
                    if q == 0:
                        sup["eps"] = pse.tile([P, SUP, P], f32, name="eps", tag="eps")
                        sup["S"] = wpool.tile([P, SUP, P], bf16, name=f"S{layer}", tag="S")
                        sup["msg"] = wpool.tile([P, SUP, P], bf16, name=f"msg{layer}", tag="msg")
                        sup["k0"] = k
                        sup["cl"] = cl
                    if cidx // ATSUP != cur_at[0]:
                        cur_at[0] = cidx // ATSUP
                        lo = cur_at[0] * ATSUP * P
                        n2 = min(ATSUP * P, NCH * P - lo)
                        at_tile[0] = stpool.tile([9, ATSUP * P], bf16, name="at", tag="at")
                        nc.sync.dma_start(out=at_tile[0][:, :n2], in_=at_d[:, lo:lo + n2])
                    if first:
                        agg = psagg.tile([P, P], f32, tag="agg")
                    a0 = (cidx - cur_at[0] * ATSUP) * P
                    if inject == "mm":
                        nc.tensor.matmul(out=sup["eps"][:, q, :],
                                         lhsT=gt[cl][:, 0, k * P:(k + 1) * P],
                                         rhs=wtab_t[:], start=True, stop=False)
                        nc.tensor.matmul(out=sup["eps"][:, q, :],
                                         lhsT=at_tile[0][:, a0:a0 + P],
                                         rhs=weaug_t[:], start=False, stop=True)
                    else:
                        nc.tensor.matmul(out=sup["eps"][:, q, :],
                                         lhsT=at_tile[0][:, a0:a0 + P],
                                         rhs=weaug_t[:], start=True, stop=True)
                    nc.vector.tensor_scalar(
                        out=sup["S"][:, q, :], in0=iotab_t[:],
                        scalar1=colrel_t[:, cidx:cidx + 1],
                        scalar2=srw_t[:, cidx:cidx + 1],
                        op0=Alu.is_equal, op1=Alu.mult)
                    pend.append((q, b, first, last, agg))
                    q += 1
                    if cidx == len(sched) - 1:
                        flush(q)

            # ---- phase 2: layer-1 pass; block finals build T2 shard ----
            def l1_block_done(b, agg):
                hb = npool.tile([P, HID], bf16, tag="hb")
                nc.scalar.activation(hb[:], agg[:], Relu,
                                     scale=dinvcol_t[:, b:b + 1])
                pst = psnode.tile([P, P], bf16, tag="pnT")
                nc.tensor.transpose(out=pst[:], in_=hb[:], identity=identb_t[:])
                hbT = npool.tile([P, P], bf16, tag="hbT")
                nc.scalar.activation(hbT[:], pst[:], Copy)
                ps2 = psnode.tile([P, P], f32, tag="pn")
                nc.tensor.matmul(out=ps2[:], lhsT=hbT[:], rhs=wcatb_t[:],
                                 start=True, stop=True)
                t2b = npool.tile([P, P], bf16, tag="t2b")
                nc.scalar.activation(t2b[:], ps2[:], Copy)
                lo = b * P
                n = min(P, SHARD - lo)
                nc.sync.dma_start(out=t2shard[lo:lo + n, :], in_=t2b[:n, :])

            edge_pass(1, xb_dram[0:TSPLIT, :], xb_dram[TSPLIT:, :],
                      W1b_t, we1b_t, l1_block_done, inject="mm")

            # ---- phase 3: one AllGather of the T2 table ----
            nc.gpsimd.collective_compute(
                "AllGather", mybir.AluOpType.bypass,
                replica_groups=[list(range(N_CORES))],
                ins=[t2shard[:]], outs=[t2full[:]])

            # ---- phase 4: layer-2/3 pass ----
            def l2_block_done(b, agg):
                w = npool.tile([P, P], f32, tag="w2")
                nc.scalar.activation(w[:], agg[:], Copy,
                                     scale=dinvcol_t[:, b:b + 1])
                lo = b * P
                n = min(P, SHARD - lo)
                nc.sync.dma_start(out=out_d[lo:lo + n, :], in_=w[:n, :])

            edge_pass(2, t2full[0:TSPLIT, :], t2full[TSPLIT:, :],
                      None, we2b_t, l2_block_done, inject="add")

    nc.finalize()
    return nc


_CACHE = {}


def kernel(**inputs):
    from concourse.bass_utils import run_bass_kernel_spmd

    per_core, runs, NCH = _host_prep(**inputs)
    key = (tuple(map(tuple, runs)), NCH)
    if key not in _CACHE:
        _CACHE[key] = _build_nc(runs, NCH)
    nc = _CACHE[key]
    r = None
    for attempt in range(3):
        try:
            r = run_bass_kernel_spmd(nc, per_core, list(range(N_CORES)))
            break
        except Exception:
            if attempt == 2:
                raise
            import time as _time
            _time.sleep(5.0)
    outs = [r.results[c]["out"] for c in range(N_CORES)]
    full = np.concatenate(outs, axis=0)
    mu = np.ascontiguousarray(full[:, :OUT_F])
    logstd = np.ascontiguousarray(full[:, OUT_F:])
    return (mu, logstd)


# revision 18
# speedup vs baseline: 1.0230x; 1.0189x over previous
"""GCN encoder (3x gcn_conv) on 8 Trainium2 NeuronCores.

Graph-parallel by destination node; per-core edges grouped into 128-node
destination blocks. Key structure (v2):

- Self-loop terms are folded in as ordinary edges: the edge-attr matrix gets a
  9th row that selects a (bias+root) row of the augmented edge weights, and
  srw=dinv[v] makes the 1/deg self scaling come out after the per-block
  dinv[col] scale.
- Node tables are bf16 in DRAM and fetched with batched dma_gather
  (transpose=True), so each 128-edge chunk's table values arrive as a
  pre-transposed [feat, edge] tile. int16 gather indices limit a table to
  32768 rows, so tables are split low/high and each block's edges are
  reordered into a low run then a high run (each padded to 128).
- Per chunk: two accumulating bf16 matmuls build relu-input PSUM
  (gathered-table @ W + at9 @ weaug), ACT applies relu into a bf16 msg tile,
  DVE builds the one-hot scatter matrix S[e,dst] = (iota==colrel)*dinv_row,
  and a third matmul accumulates S^T @ msg into the block's PSUM agg.
- Block final: relu(dinv[col] * agg) in one ACT op -> h block; T2 = h @
  [Wmu|Wls] per own block; ONE bf16 AllGather of T2 between the passes.
  Layer-2 repeats the edge pass over the T2 table (identity rhs) and writes
  dinv[col] * agg2 as the output.
"""
import numpy as np

N_NODES = 50000
N_CORES = 8
SHARD = N_NODES // N_CORES          # 6250
P = 128
NBLK = (SHARD + P - 1) // P         # 49 destination blocks / core
IN_F = 128
HID = 128
OUT_F = 64
TSPLIT = 32768                      # int16 gather index limit
TBLK = (N_NODES + P - 1) // P       # 391 x-cast chunks


def _host_prep(x, edge_index, edge_attr,
               W1, b1, We1, be1, root1,
               Wmu, bmu, Wemu, bemu, rootmu,
               Wls, bls, Wels, bels, rootls):
    x = np.asarray(x, np.float32)
    row = np.asarray(edge_index[0], np.int64)
    col = np.asarray(edge_index[1], np.int64)
    ea = np.asarray(edge_attr, np.float32)
    E = row.shape[0]

    deg = (np.bincount(row, minlength=N_NODES) + 1.0).astype(np.float32)
    dinv = deg ** -0.5

    # append self-edges (v -> v); at row 8 marks them
    selfv = np.arange(N_NODES, dtype=np.int64)
    row_a = np.concatenate([row, selfv])
    col_a = np.concatenate([col, selfv])
    is_self = np.concatenate([np.zeros(E, bool), np.ones(N_NODES, bool)])
    EA = row_a.shape[0]

    core_of = col_a // SHARD
    blk_of = (col_a - core_of * SHARD) // P
    cls_of = (row_a >= TSPLIT).astype(np.int64)   # 0 = low table, 1 = high

    # uniform chunks per (block, class) across cores
    counts = np.zeros((N_CORES, NBLK, 2), np.int64)
    for c in range(N_CORES):
        m = core_of == c
        np.add.at(counts[c], (blk_of[m], cls_of[m]), 1)
    ncls = (counts.max(axis=0) + P - 1) // P      # [NBLK, 2]
    NCH = int(ncls.sum())

    # chunk-column base for each (block, class) run; runs laid out
    # block-major, low run then high run
    run_base = np.zeros((NBLK, 2), np.int64)
    acc = 0
    runs = []          # (b, cls, nch, chunk_base)
    for b in range(NBLK):
        for cl in range(2):
            run_base[b, cl] = acc
            if ncls[b, cl] > 0:
                runs.append((b, cl, int(ncls[b, cl]), acc))
            acc += int(ncls[b, cl])

    colrel = np.full((N_CORES, P, NCH), -1.0, np.float32)
    srw = np.zeros((N_CORES, P, NCH), np.float32)
    at = np.zeros((N_CORES, 9, NCH * P), np.float32)
    gidx = np.zeros((N_CORES, P, NCH * 8), np.int16)

    order = np.lexsort((cls_of, blk_of, core_of))
    row_s, col_s = row_a[order], col_a[order]
    core_s, blk_s, cls_s = core_of[order], blk_of[order], cls_of[order]
    self_s = is_self[order]
    ea_idx = order  # indices into the augmented edge list; < E means real

    seg_cnt = np.zeros(N_CORES * NBLK * 2 + 1, np.int64)
    seg_key = (core_s * NBLK + blk_s) * 2 + cls_s
    np.add.at(seg_cnt, seg_key + 1, 1)
    seg_start = np.cumsum(seg_cnt)
    pos_in_seg = np.arange(EA) - seg_start[seg_key]

    chunk_idx = run_base[blk_s, cls_s] + pos_in_seg // P
    part_idx = pos_in_seg % P

    colrel[core_s, part_idx, chunk_idx] = (col_s - core_s * SHARD - blk_s * P).astype(np.float32)
    srw[core_s, part_idx, chunk_idx] = dinv[row_s]
    flat = chunk_idx * P + part_idx
    real = ~self_s
    for j in range(7):
        at[core_s[real], j, flat[real]] = ea[ea_idx[real], j]
    at[core_s[real], 7, flat[real]] = 1.0
    at[core_s[self_s], 8, flat[self_s]] = 1.0

    # gather indices, rebased per table half, wrapped-16 layout replicated x8:
    # index position i within a run -> [16g + i%16, run_chunk_base*8 + i//16]
    rel_row = (row_s - cls_s * TSPLIT).astype(np.int16)
    gcol = run_base[blk_s, cls_s] * 8 + pos_in_seg // 16
    gpart = pos_in_seg % 16
    for g in range(8):
        gidx[core_s, 16 * g + gpart, gcol] = rel_row

    dinvcol = np.zeros((N_CORES, P, NBLK), np.float32)
    for c in range(N_CORES):
        ids = c * SHARD + np.arange(SHARD)
        b = np.arange(SHARD) // P
        p = np.arange(SHARD) % P
        dinvcol[c, p, b] = dinv[ids]

    def bf(a):
        import ml_dtypes
        return np.asarray(a, np.float32).astype(ml_dtypes.bfloat16)

    weaug1 = np.concatenate([np.asarray(We1, np.float32),
                             (np.asarray(be1) + np.asarray(b1))[None, :],
                             (np.asarray(b1) + np.asarray(root1))[None, :]], 0)
    weaug2 = np.concatenate([
        np.concatenate([np.asarray(Wemu), np.asarray(Wels)], 1),
        np.concatenate([np.asarray(bemu) + np.asarray(bmu),
                        np.asarray(bels) + np.asarray(bls)])[None, :],
        np.concatenate([np.asarray(bmu) + np.asarray(rootmu),
                        np.asarray(bls) + np.asarray(rootls)])[None, :]], 0)
    wcat = np.concatenate([np.asarray(Wmu), np.asarray(Wls)], 1)
    iota = np.tile(np.arange(P, dtype=np.float32)[None, :], (P, 1))
    ident = np.eye(P, dtype=np.float32)

    shared = dict(xrows=np.ascontiguousarray(x), W1b=bf(W1), wcatb=bf(wcat),
                  we1b=bf(weaug1), we2b=bf(weaug2), iotab=bf(iota),
                  identb=bf(ident))
    per_core = []
    for c in range(N_CORES):
        d = dict(colrel=colrel[c], srw=srw[c], at=bf(at[c]), gidx=gidx[c],
                 dinvcol=dinvcol[c])
        d.update(shared)
        per_core.append(d)
    return per_core, runs, NCH


def _build_nc(runs, NCH):
    from concourse import bass, bacc, mybir
    import concourse.tile as tile

    f32 = mybir.dt.float32
    bf16 = mybir.dt.bfloat16
    i16 = mybir.dt.int16
    Relu = mybir.ActivationFunctionType.Relu
    Copy = mybir.ActivationFunctionType.Copy
    Alu = mybir.AluOpType
    nc = bacc.Bacc(None, num_devices=N_CORES)

    xrows_d = nc.declare_dram_parameter("xrows", [N_NODES, IN_F], f32, isOutput=False)
    W1b_d = nc.declare_dram_parameter("W1b", [IN_F, HID], bf16, isOutput=False)
    wcatb_d = nc.declare_dram_parameter("wcatb", [HID, P], bf16, isOutput=False)
    we1b_d = nc.declare_dram_parameter("we1b", [9, HID], bf16, isOutput=False)
    we2b_d = nc.declare_dram_parameter("we2b", [9, P], bf16, isOutput=False)
    iotab_d = nc.declare_dram_parameter("iotab", [P, P], bf16, isOutput=False)
    identb_d = nc.declare_dram_parameter("identb", [P, P], bf16, isOutput=False)
    colrel_d = nc.declare_dram_parameter("colrel", [P, NCH], f32, isOutput=False)
    srw_d = nc.declare_dram_parameter("srw", [P, NCH], f32, isOutput=False)
    at_d = nc.declare_dram_parameter("at", [9, NCH * P], bf16, isOutput=False)
    gidx_d = nc.declare_dram_parameter("gidx", [P, NCH * 8], i16, isOutput=False)
    dinvcol_d = nc.declare_dram_parameter("dinvcol", [P, NBLK], f32, isOutput=False)
    out_d = nc.declare_dram_parameter("out", [SHARD, P], f32, isOutput=True)

    xb_dram = nc.dram_tensor("xb", [N_NODES, IN_F], bf16)
    t2shard = nc.dram_tensor("t2shard", [SHARD, P], bf16)
    t2full = nc.dram_tensor("t2full", [N_NODES, P], bf16, addr_space="Shared")

    SUP = 4      # chunks per relu batch
    ATSUP = 64   # chunks per at-stream tile
    MAXRUN = max(n for (_b, _cl, n, _cb) in runs)

    # per-chunk schedule: (b, cls, k_in_run, run_chunk_base, first, last)
    sched = []
    for b in range(NBLK):
        blk_runs = [r for r in runs if r[0] == b]
        tot = sum(r[2] for r in blk_runs)
        ki = 0
        for (_b, cl, n, cb) in blk_runs:
            for k in range(n):
                sched.append((b, cl, k, cb, ki == 0, ki == tot - 1))
                ki += 1

    with tile.TileContext(nc) as tc:
        with (
            tc.tile_pool(name="const", bufs=1) as cpool,
            tc.tile_pool(name="stream", bufs=2) as stpool,
            tc.tile_pool(name="gat", bufs=2) as gpool,
            tc.tile_pool(name="work", bufs=3) as wpool,
            tc.tile_pool(name="node", bufs=3) as npool,
            tc.tile_pool(name="cast", bufs=4) as xpool,
            tc.tile_pool(name="pse", bufs=3, space="PSUM") as pse,
            tc.tile_pool(name="psagg", bufs=2, space="PSUM") as psagg,
            tc.tile_pool(name="psnode", bufs=1, space="PSUM") as psnode,
        ):
            W1b_t = cpool.tile([IN_F, HID], bf16)
            wcatb_t = cpool.tile([HID, P], bf16)
            we1b_t = cpool.tile([9, HID], bf16)
            we2b_t = cpool.tile([9, P], bf16)
            iotab_t = cpool.tile([P, P], bf16)
            identb_t = cpool.tile([P, P], bf16)
            colrel_t = cpool.tile([P, NCH], f32)
            srw_t = cpool.tile([P, NCH], f32)
            gidx_t = cpool.tile([P, NCH * 8], i16)
            dinvcol_t = cpool.tile([P, NBLK], f32)
            for t, d in ((W1b_t, W1b_d), (wcatb_t, wcatb_d), (we1b_t, we1b_d),
                         (we2b_t, we2b_d), (iotab_t, iotab_d),
                         (identb_t, identb_d), (colrel_t, colrel_d),
                         (srw_t, srw_d), (gidx_t, gidx_d),
                         (dinvcol_t, dinvcol_d)):
                nc.sync.dma_start(out=t[:], in_=d[:])

            # ---- phase 1: cast x -> bf16 table in DRAM (8 chunks per DMA) ----
            CB = 8
            for tb in range(0, TBLK, CB):
                nchk = min(CB, TBLK - tb)
                lo = tb * P
                n = min(nchk * P, N_NODES - lo)
                xt = xpool.tile([P, CB, IN_F], f32, tag="xt")
                xbt = xpool.tile([P, CB, IN_F], bf16, tag="xbt")
                full = n // P          # whole chunks in this batch
                if full:
                    xv = xrows_d[lo:lo + full * P, :].rearrange("(c p) f -> p c f", p=P)
                    nc.sync.dma_start(out=xt[:, :full, :], in_=xv)
                    nc.vector.tensor_copy(out=xbt[:, :full, :], in_=xt[:, :full, :])
                    nc.sync.dma_start(
                        out=xb_dram[lo:lo + full * P, :].rearrange("(c p) f -> p c f", p=P),
                        in_=xbt[:, :full, :])
                rem = n - full * P     # trailing partial chunk (last batch)
                if rem:
                    xr = xpool.tile([P, IN_F], f32, tag="xtr")
                    nc.sync.dma_start(out=xr[:rem, :], in_=xrows_d[lo + full * P:lo + n, :])
                    xbr = xpool.tile([P, IN_F], bf16, tag="xbr")
                    nc.vector.tensor_copy(out=xbr[:rem, :], in_=xr[:rem, :])
                    nc.sync.dma_start(out=xb_dram[lo + full * P:lo + n, :], in_=xbr[:rem, :])

            # ---- edge pass ----
            # inject="mm": transposed gather + per-chunk table matmul (l1)
            # inject="add": plain gather + one wide DVE add per super (l2,
            #   whose table T2 is premultiplied so only the at-term needs PE)
            def edge_pass(layer, tab_lo, tab_hi, wtab_t, weaug_t, block_done,
                          inject):
                at_tile = [None]
                cur_at = [-1]
                sup = {}
                pend = []
                gt = {}

                def flush(nq):
                    if inject == "add":
                        pre = wpool.tile([P, SUP, P], f32, name="pre", tag="pre")
                        k0 = sup["k0"]
                        nc.vector.tensor_tensor(
                            out=pre[:, :nq, :],
                            in0=gt[sup["cl"]][:, k0:k0 + nq, :],
                            in1=sup["eps"][:, :nq, :], op=Alu.add)
                        nc.scalar.activation(sup["msg"][:, :nq, :],
                                             pre[:, :nq, :], Relu)
                    else:
                        nc.scalar.activation(sup["msg"][:, :nq, :],
                                             sup["eps"][:, :nq, :], Relu)
                    for (qq, bb, first, last, agg) in pend:
                        nc.tensor.matmul(
                            out=agg[:], lhsT=sup["S"][:, qq, :],
                            rhs=sup["msg"][:, qq, :],
                            start=first, stop=last)
                        if last:
                            block_done(bb, agg)
                    pend.clear()

                agg = None
                q = 0
                for cidx, (b, cl, k, cb, first, last) in enumerate(sched):
                    if q == SUP or (inject == "add" and k == 0 and q > 0):
                        flush(q)
                        q = 0
                    if k == 0:
                        # new run: batched gathers (the SWDGE ring wedges
                        # beyond 896 indices per instruction)
                        GMAX = 7
                        n = next(r[2] for r in runs if r[0] == b and r[1] == cl)
                        if inject == "mm":
                            gt[cl] = gpool.tile([P, 1, MAXRUN * P], bf16,
                                                name=f"g{layer}c{cl}", tag=f"g{cl}")
                        else:
                            gt[cl] = gpool.tile([P, MAXRUN, P], bf16,
                                                name=f"g{layer}c{cl}", tag=f"g{cl}")
                        for s0 in range(0, n, GMAX):
                            sn = min(GMAX, n - s0)
                            ni = sn * P
                            out_ap = (gt[cl][:, :, s0 * P:s0 * P + ni]
                                      if inject == "mm"
                                      else gt[cl][:, s0:s0 + sn, :])
                            nc.gpsimd.dma_gather(
                                out_ap=out_ap,
                                in_ap=tab_lo[:] if cl == 0 else tab_hi[:],
                                idxs_ap=gidx_t[:, (cb + s0) * 8:(cb + s0) * 8 + ni // 16],
                                num_idxs=ni, num_idxs_reg=ni,
                                elem_size=P, transpose=(inject == "mm"))
                    if q == 0:
                        sup["eps"] = pse.tile([P, SUP, P], f32, name="eps", tag="eps")
                        sup["S"] = wpool.tile([P, SUP, P], bf16, name=f"S{layer}", tag="S")
                        sup["msg"] = wpool.tile([P, SUP, P], bf16, name=f"msg{layer}", tag="msg")
                        sup["k0"] = k
                        sup["cl"] = cl
                    if cidx // ATSUP != cur_at[0]:
                        cur_at[0] = cidx // ATSUP
                        lo = cur_at[0] * ATSUP * P
                        n2 = min(ATSUP * P, NCH * P - lo)
                        at_tile[0] = stpool.tile([9, ATSUP * P], bf16, name="at", tag="at")
                        nc.sync.dma_start(out=at_tile[0][:, :n2], in_=at_d[:, lo:lo + n2])
                    if first:
                        agg = psagg.tile([P, P], f32, tag="agg")
                    a0 = (cidx - cur_at[0] * ATSUP) * P
                    if inject == "mm":
                        nc.tensor.matmul(out=sup["eps"][:, q, :],
                                         lhsT=gt[cl][:, 0, k * P:(k + 1) * P],
                                         rhs=wtab_t[:], start=True, stop=False)
                        nc.tensor.matmul(out=sup["eps"][:, q, :],
                                         lhsT=at_tile[0][:, a0:a0 + P],
                                         rhs=weaug_t[:], start=False, stop=True)
                    else:
                        nc.tensor.matmul(out=sup["eps"][:, q, :],
                                         lhsT=at_tile[0][:, a0:a0 + P],
                                         rhs=weaug_t[:], start=True, stop=True)
                    nc.vector.tensor_scalar(
                        out=sup["S"][:, q, :], in0=iotab_t[:],
                        scalar1=colrel_t[:, cidx:cidx + 1],
                        scalar2=srw_t[:, cidx:cidx + 1],
                        op0=Alu.is_equal, op1=Alu.mult)
                    pend.append((q, b, first, last, agg))
                    q += 1
                    if cidx == len(sched) - 1:
                        flush(q)

            # ---- phase 2: layer-1 pass; block finals build T2 shard ----
            def l1_block_done(b, agg):
                hb = npool.tile([P, HID], bf16, tag="hb")
                nc.scalar.activation(hb[:], agg[:], Relu,
                                     scale=dinvcol_t[:, b:b + 1])
                pst = psnode.tile([P, P], bf16, tag="pnT")
                nc.tensor.transpose(out=pst[:], in_=hb[:], identity=identb_t[:])
                hbT = npool.tile([P, P], bf16, tag="hbT")
                nc.scalar.activation(hbT[:], pst[:], Copy)
                ps2 = psnode.tile([P, P], f32, tag="pn")
                nc.tensor.matmul(out=ps2[:], lhsT=hbT[:], rhs=wcatb_t[:],
                                 start=True, stop=True)
                t2b = npool.tile([P, P], bf16, tag="t2b")
                nc.scalar.activation(t2b[:], ps2[:], Copy)
                lo = b * P
                n = min(P, SHARD - lo)
                nc.sync.dma_start(out=t2shard[lo:lo + n, :], in_=t2b[:n, :])

            edge_pass(1, xb_dram[0:TSPLIT, :], xb_dram[TSPLIT:, :],
                      W1b_t, we1b_t, l1_block_done, inject="mm")

            # ---- phase 3: one AllGather of the T2 table ----
            nc.gpsimd.collective_compute(
                "AllGather", mybir.AluOpType.bypass,
                replica_groups=[list(range(N_CORES))],
                ins=[t2shard[:]], outs=[t2full[:]])

            # ---- phase 4: layer-2/3 pass ----
            def l2_block_done(b, agg):
                w = npool.tile([P, P], f32, tag="w2")
                nc.scalar.activation(w[:], agg[:], Copy,
                                     scale=dinvcol_t[:, b:b + 1])
                lo = b * P
                n = min(P, SHARD - lo)
                nc.sync.dma_start(out=out_d[lo:lo + n, :], in_=w[:n, :])

            edge_pass(2, t2full[0:TSPLIT, :], t2full[TSPLIT:, :],
                      None, we2b_t, l2_block_done, inject="add")

    nc.finalize()
    return nc


_CACHE = {}


def kernel(**inputs):
    from concourse.bass_utils import run_bass_kernel_spmd

    per_core, runs, NCH = _host_prep(**inputs)
    key = (tuple(map(tuple, runs)), NCH)
    if key not in _CACHE:
        _CACHE[key] = _build_nc(runs, NCH)
    nc = _CACHE[key]
    r = None
    for attempt in range(3):
        try:
            r = run_bass_kernel_spmd(nc, per_core, list(range(N_CORES)))
            break
        except Exception:
            if attempt == 2:
                raise
            import time as _time
            _time.sleep(5.0)
    outs = [r.results[c]["out"] for c in range(N_CORES)]
    full = np.concatenate(outs, axis=0)
    mu = np.ascontiguousarray(full[:, :OUT_F])
    logstd = np.ascontiguousarray(full[:, OUT_F:])
    return (mu, logstd)
